# revision 6
# baseline (speedup 1.0000x reference)
"""Distributed TRN2 kernel for nn_CustomFullyConnectedLayerSoftmax.

Math: the reference's scatter-add builds W[r, c] = V_scaled[(r-c) % 2048, c]
(each (r, c) hit exactly once -> pure permutation), then out = x @ W.T.
So out[:, r] needs column r of W.T, i.e. W.T[c, r] = V_scaled[(r-c)%2048, c].

Sharding: output columns r are split across 8 cores (256 each). Core i
receives B_i = W.T[:, 256*i : 256*(i+1)] as a dense [2048, 256] operand,
interleaved with the replicated x.T into a single input tensor laid out in
SBUF geometry: IN[p, k, 0:32] = x.T[k*128+p, :], IN[p, k, 32:288] =
B_i[k*128+p, :]. Each core computes its disjoint out[:, 256*i:256*(i+1)] =
x @ B_i with 16 accumulating matmuls -- no collectives; host concatenates
the 8 slices.

Device traffic per core: its 1/8 share of V plus a replicated x -- the
memory roofline for this op.
"""

import numpy as np

from concourse import bass, bacc, mybir, tile
from concourse import bass_utils

IN_F = 2048
OUT_F = 2048
TOTAL = 2048
BATCH = 32
N_CORES = 8
R_SH = OUT_F // N_CORES          # 256 output columns per core
K_CH = IN_F // 128               # 16 contraction chunks of 128
W_CH = BATCH + R_SH              # 288 = interleaved xT + B row width
K_TOPK = 1844                    # ceil(int(0.9 * 2048 * 2048) / 2048)

# 'f32' or 'bf16' compute/storage dtype for the matmul operands.
DEVICE_DTYPE = "bf16"
# Chunks the load+matmul pipeline is split into (must divide K_CH).
N_SPLITS = 4
# True: raw hand-scheduled bacc kernel; False: Tile-scheduled kernel.
RAW = True

TRACE = False          # set True (from test.py) to capture neuron-profile
TRACE_KWARGS = {}
LAST_RESULT = None     # BassKernelResults of the most recent run

_graph_cache = {}


def _mybir_dt(key):
    return mybir.dt.float32 if key == "f32" else mybir.dt.bfloat16


def _np_dt(key):
    return mybir.dt.np(_mybir_dt(key))


def _build_graph_tile(dtype_key):
    dt = _mybir_dt(dtype_key)
    nc = bacc.Bacc("TRN2", target_bir_lowering=False, debug=False,
                   enable_asserts=False)

    in_d = nc.dram_tensor("IN", [128, K_CH, W_CH], dt, kind="ExternalInput")
    out_d = nc.dram_tensor("out", [BATCH, R_SH], mybir.dt.float32,
                           kind="ExternalOutput")

    kper = K_CH // N_SPLITS
    dma_engines = [nc.sync, nc.scalar]
    with tile.TileContext(nc) as tc:
        with (
            tc.tile_pool(name="inpool", bufs=N_SPLITS) as inpool,
            tc.tile_pool(name="opool", bufs=1) as opool,
            tc.tile_pool(name="psum", bufs=1, space="PSUM") as pspool,
        ):
            acc = pspool.tile([BATCH, R_SH], mybir.dt.float32)
            tiles = []
            for j in range(N_SPLITS):
                t = inpool.tile([128, kper, W_CH], dt, tag="in")
                dma_engines[j % 2].dma_start(
                    t[:], in_d[:, j * kper:(j + 1) * kper, :])
                tiles.append(t)
            for j in range(N_SPLITS):
                for k in range(kper):
                    kk = j * kper + k
                    nc.tensor.matmul(
                        acc[:],
                        tiles[j][:, k, 0:BATCH],
                        tiles[j][:, k, BATCH:W_CH],
                        start=(kk == 0),
                        stop=(kk == K_CH - 1),
                    )
            ot = opool.tile([BATCH, R_SH], mybir.dt.float32)
            nc.vector.tensor_copy(ot[:], acc[:])
            nc.sync.dma_start(out_d[:], ot[:])

    nc.compile()
    return nc


# k-slice counts per pipelined chunk (must sum to K_CH). Small first chunk
# gets the PE started early; small last chunk minimizes the matmul tail
# exposed after the final DMA-completion semaphore.
CHUNKS = [1, 5, 5, 5]


def _build_graph_raw(dtype_key):
    dt = _mybir_dt(dtype_key)
    nc = bass.Bass("TRN2", target_bir_lowering=False, debug=False,
                   enable_asserts=False)

    in_d = nc.dram_tensor("IN", [128, K_CH, W_CH], dt, kind="ExternalInput")
    out_d = nc.dram_tensor("out", [BATCH, R_SH], mybir.dt.float32,
                           kind="ExternalOutput")

    assert sum(CHUNKS) == K_CH
    bounds = [0]
    for c in CHUNKS:
        bounds.append(bounds[-1] + c)
    # chunk j -> (engine index, cumulative DMA count on that engine)
    eng_of = [j % 2 for j in range(len(CHUNKS))]
    cum = []
    counts = [0, 0]
    for j in range(len(CHUNKS)):
        counts[eng_of[j]] += 1
        cum.append(counts[eng_of[j]])

    with (
        nc.semaphore("sem_s") as sem_s,      # sync-issued DMA completions
        nc.semaphore("sem_a") as sem_a,      # scalar-issued DMA completions
        nc.semaphore("msem") as msem,
        nc.semaphore("csem") as csem,
        nc.sbuf_tensor("inb", [128, K_CH, W_CH], dt) as inb,
        nc.psum_tensor("acc", [BATCH, R_SH], mybir.dt.float32) as acc,
        nc.sbuf_tensor("ot", [BATCH, R_SH], mybir.dt.float32) as ot,
        nc.Block() as block,
    ):
        sems = [sem_s, sem_a]

        # Even chunks stream through sync's HWDGE ring, odd ones through
        # scalar's -- two FIFOs, each with cumulative +16 semaphores.
        @block.sync
        def _(sync):
            for j in range(len(CHUNKS)):
                if eng_of[j] == 0:
                    sync.dma_start(
                        inb[:, bounds[j]:bounds[j + 1], :],
                        in_d[:, bounds[j]:bounds[j + 1], :],
                    ).then_inc(sem_s, 16)
            sync.wait_ge(csem, 1)
            # No wait after this: the data lands during the NEFF teardown
            # barrier chain, long before the host reads the output.
            sync.dma_start(out_d[:, :], ot[:, :]).then_inc(sem_s, 16)

        @block.scalar
        def _(scalar):
            for j in range(len(CHUNKS)):
                if eng_of[j] == 1:
                    scalar.dma_start(
                        inb[:, bounds[j]:bounds[j + 1], :],
                        in_d[:, bounds[j]:bounds[j + 1], :],
                    ).then_inc(sem_a, 16)

        @block.tensor
        def _(tensor):
            for j in range(len(CHUNKS)):
                tensor.wait_ge(sems[eng_of[j]], 16 * cum[j])
                for kk in range(bounds[j], bounds[j + 1]):
                    mm = tensor.matmul(
                        acc[:, :],
                        inb[:, kk, 0:BATCH],
                        inb[:, kk, BATCH:W_CH],
                        start=(kk == 0),
                        stop=(kk == K_CH - 1),
                    )
            mm.then_inc(msem, 1)

        @block.vector
        def _(vector):
            vector.wait_ge(msem, 1)
            vector.tensor_copy(ot[:, :], acc[:, :]).then_inc(csem, 1)

    return nc


def _get_graph(dtype_key):
    key = (dtype_key, RAW, N_SPLITS)
    if key not in _graph_cache:
        build = _build_graph_raw if RAW else _build_graph_tile
        _graph_cache[key] = build(dtype_key)
    return _graph_cache[key]


def _host_shards(x, V, alpha, dtype_key):
    np_dt = _np_dt(dtype_key)

    a = alpha.astype(np.float64)
    e = np.exp(a - a.max())
    scale = np.clip(K_TOPK * (e / e.sum()), 0.0, 1.0).astype(np.float32)
    Vs = V * scale[:, None]                        # [2048, 2048] f32

    # W.T[c, r] = Vs[(r - c) % 2048, c]; with Vt = Vs.T duplicated along
    # columns, row c of W.T is the window Vt2[c, 2048-c : 4096-c] -> a
    # shear expressible as a strided view of the flat buffer.
    Vt2 = np.concatenate([Vs.T, Vs.T], axis=1)     # [2048, 4096]
    flat = np.ascontiguousarray(Vt2).reshape(-1)
    WT = np.lib.stride_tricks.as_strided(
        flat[TOTAL:], shape=(IN_F, OUT_F),
        strides=((2 * TOTAL - 1) * 4, 4))

    xT = np.ascontiguousarray(x.T)                 # [2048, 32]
    # [128, K_CH, BATCH]
    xT_dev = xT.reshape(K_CH, 128, BATCH).transpose(1, 0, 2)

    in_maps = []
    for i in range(N_CORES):
        Bi = np.asarray(WT[:, i * R_SH:(i + 1) * R_SH])   # [2048, 256]
        Bi_dev = Bi.reshape(K_CH, 128, R_SH).transpose(1, 0, 2)
        merged = np.empty((128, K_CH, W_CH), dtype=np_dt)
        merged[:, :, :BATCH] = xT_dev
        merged[:, :, BATCH:] = Bi_dev
        in_maps.append({"IN": merged})
    return in_maps


def kernel(x, V, alpha):
    global LAST_RESULT
    x = np.asarray(x, dtype=np.float32)
    V = np.asarray(V, dtype=np.float32)
    alpha = np.asarray(alpha, dtype=np.float32)

    in_maps = _host_shards(x, V, alpha, DEVICE_DTYPE)
    nc = _get_graph(DEVICE_DTYPE)
    res = bass_utils.run_bass_kernel_spmd(
        nc, in_maps, core_ids=list(range(N_CORES)),
        trace=TRACE, trace_kwargs=TRACE_KWARGS)
    LAST_RESULT = res
    out = np.concatenate([np.asarray(r["out"]) for r in res.results], axis=1)
    return np.ascontiguousarray(out, dtype=np.float32)


# revision 7
# speedup vs baseline: 1.1402x; 1.1402x over previous
"""Distributed TRN2 kernel for nn_CustomFullyConnectedLayerSoftmax.

Math: the reference's scatter-add builds W[r, c] = V_scaled[(r-c) % 2048, c]
(each (r, c) hit exactly once -> pure permutation), then out = x @ W.T.
So out[:, r] needs column r of W.T, i.e. W.T[c, r] = V_scaled[(r-c)%2048, c].

Sharding: output columns r are split across 8 cores (256 each). Core i
receives B_i = W.T[:, 256*i : 256*(i+1)] as a dense [2048, 256] operand,
interleaved with the replicated x.T into a single input tensor laid out in
SBUF geometry: IN[p, k, 0:32] = x.T[k*128+p, :], IN[p, k, 32:288] =
B_i[k*128+p, :]. Each core computes its disjoint out[:, 256*i:256*(i+1)] =
x @ B_i with 16 accumulating matmuls -- no collectives; host concatenates
the 8 slices.

Device traffic per core: its 1/8 share of V plus a replicated x -- the
memory roofline for this op.
"""

import numpy as np

from concourse import bass, bacc, mybir, tile
from concourse import bass_utils

IN_F = 2048
OUT_F = 2048
TOTAL = 2048
BATCH = 32
N_CORES = 8
R_SH = OUT_F // N_CORES          # 256 output columns per core
K_CH = IN_F // 128               # 16 contraction chunks of 128
W_CH = BATCH + R_SH              # 288 = interleaved xT + B row width
K_TOPK = 1844                    # ceil(int(0.9 * 2048 * 2048) / 2048)

# 'f32' or 'bf16' compute/storage dtype for the matmul operands.
DEVICE_DTYPE = "bf16"
# Chunks the load+matmul pipeline is split into (must divide K_CH).
N_SPLITS = 4
# True: raw hand-scheduled bacc kernel; False: Tile-scheduled kernel.
RAW = True

TRACE = False          # set True (from test.py) to capture neuron-profile
TRACE_KWARGS = {}
LAST_RESULT = None     # BassKernelResults of the most recent run

_graph_cache = {}


def _mybir_dt(key):
    return mybir.dt.float32 if key == "f32" else mybir.dt.bfloat16


def _np_dt(key):
    return mybir.dt.np(_mybir_dt(key))


def _build_graph_tile(dtype_key):
    dt = _mybir_dt(dtype_key)
    nc = bacc.Bacc("TRN2", target_bir_lowering=False, debug=False,
                   enable_asserts=False)

    in_d = nc.dram_tensor("IN", [128, K_CH, W_CH], dt, kind="ExternalInput")
    out_d = nc.dram_tensor("out", [BATCH, R_SH], mybir.dt.float32,
                           kind="ExternalOutput")

    kper = K_CH // N_SPLITS
    dma_engines = [nc.sync, nc.scalar]
    with tile.TileContext(nc) as tc:
        with (
            tc.tile_pool(name="inpool", bufs=N_SPLITS) as inpool,
            tc.tile_pool(name="opool", bufs=1) as opool,
            tc.tile_pool(name="psum", bufs=1, space="PSUM") as pspool,
        ):
            acc = pspool.tile([BATCH, R_SH], mybir.dt.float32)
            tiles = []
            for j in range(N_SPLITS):
                t = inpool.tile([128, kper, W_CH], dt, tag="in")
                dma_engines[j % 2].dma_start(
                    t[:], in_d[:, j * kper:(j + 1) * kper, :])
                tiles.append(t)
            for j in range(N_SPLITS):
                for k in range(kper):
                    kk = j * kper + k
                    nc.tensor.matmul(
                        acc[:],
                        tiles[j][:, k, 0:BATCH],
                        tiles[j][:, k, BATCH:W_CH],
                        start=(kk == 0),
                        stop=(kk == K_CH - 1),
                    )
            ot = opool.tile([BATCH, R_SH], mybir.dt.float32)
            nc.vector.tensor_copy(ot[:], acc[:])
            nc.sync.dma_start(out_d[:], ot[:])

    nc.compile()
    return nc


# k-slice counts per pipelined chunk (must sum to K_CH). Small first chunk
# gets the PE started early; small last chunk minimizes the matmul tail
# exposed after the final DMA-completion semaphore.
CHUNKS = [5, 5, 5, 1]


def _build_graph_raw(dtype_key):
    dt = _mybir_dt(dtype_key)
    nc = bass.Bass("TRN2", target_bir_lowering=False, debug=False,
                   enable_asserts=False)

    in_d = nc.dram_tensor("IN", [128, K_CH, W_CH], dt, kind="ExternalInput")
    out_d = nc.dram_tensor("out", [BATCH, R_SH], mybir.dt.float32,
                           kind="ExternalOutput")

    assert sum(CHUNKS) == K_CH
    bounds = [0]
    for c in CHUNKS:
        bounds.append(bounds[-1] + c)
    # chunk j -> (engine index, cumulative DMA count on that engine)
    eng_of = [j % 2 for j in range(len(CHUNKS))]
    cum = []
    counts = [0, 0]
    for j in range(len(CHUNKS)):
        counts[eng_of[j]] += 1
        cum.append(counts[eng_of[j]])

    with (
        nc.semaphore("sem_s") as sem_s,      # sync-issued DMA completions
        nc.semaphore("sem_a") as sem_a,      # scalar-issued DMA completions
        nc.semaphore("msem") as msem,
        nc.semaphore("csem") as csem,
        nc.sbuf_tensor("inb", [128, K_CH, W_CH], dt) as inb,
        nc.psum_tensor("acc", [BATCH, R_SH], mybir.dt.float32) as acc,
        nc.sbuf_tensor("ot", [BATCH, R_SH], mybir.dt.float32) as ot,
        nc.Block() as block,
    ):
        sems = [sem_s, sem_a]

        # Even chunks stream through sync's HWDGE ring, odd ones through
        # scalar's -- two FIFOs, each with cumulative +16 semaphores.
        @block.sync
        def _(sync):
            for j in range(len(CHUNKS)):
                if eng_of[j] == 0:
                    sync.dma_start(
                        inb[:, bounds[j]:bounds[j + 1], :],
                        in_d[:, bounds[j]:bounds[j + 1], :],
                    ).then_inc(sem_s, 16)
            sync.wait_ge(csem, 1)
            # No wait after this: the data lands during the NEFF teardown
            # barrier chain, long before the host reads the output.
            sync.dma_start(out_d[:, :], ot[:, :]).then_inc(sem_s, 16)

        @block.scalar
        def _(scalar):
            for j in range(len(CHUNKS)):
                if eng_of[j] == 1:
                    scalar.dma_start(
                        inb[:, bounds[j]:bounds[j + 1], :],
                        in_d[:, bounds[j]:bounds[j + 1], :],
                    ).then_inc(sem_a, 16)

        @block.tensor
        def _(tensor):
            for j in range(len(CHUNKS)):
                tensor.wait_ge(sems[eng_of[j]], 16 * cum[j])
                for kk in range(bounds[j], bounds[j + 1]):
                    mm = tensor.matmul(
                        acc[:, :],
                        inb[:, kk, 0:BATCH],
                        inb[:, kk, BATCH:W_CH],
                        start=(kk == 0),
                        stop=(kk == K_CH - 1),
                    )
            mm.then_inc(msem, 1)

        @block.vector
        def _(vector):
            vector.wait_ge(msem, 1)
            vector.tensor_copy(ot[:, :], acc[:, :]).then_inc(csem, 1)

    return nc


def _get_graph(dtype_key):
    key = (dtype_key, RAW, N_SPLITS)
    if key not in _graph_cache:
        build = _build_graph_raw if RAW else _build_graph_tile
        _graph_cache[key] = build(dtype_key)
    return _graph_cache[key]


def _host_shards(x, V, alpha, dtype_key):
    np_dt = _np_dt(dtype_key)

    a = alpha.astype(np.float64)
    e = np.exp(a - a.max())
    scale = np.clip(K_TOPK * (e / e.sum()), 0.0, 1.0).astype(np.float32)
    Vs = V * scale[:, None]                        # [2048, 2048] f32

    # W.T[c, r] = Vs[(r - c) % 2048, c]; with Vt = Vs.T duplicated along
    # columns, row c of W.T is the window Vt2[c, 2048-c : 4096-c] -> a
    # shear expressible as a strided view of the flat buffer.
    Vt2 = np.concatenate([Vs.T, Vs.T], axis=1)     # [2048, 4096]
    flat = np.ascontiguousarray(Vt2).reshape(-1)
    WT = np.lib.stride_tricks.as_strided(
        flat[TOTAL:], shape=(IN_F, OUT_F),
        strides=((2 * TOTAL - 1) * 4, 4))

    xT = np.ascontiguousarray(x.T)                 # [2048, 32]
    # [128, K_CH, BATCH]
    xT_dev = xT.reshape(K_CH, 128, BATCH).transpose(1, 0, 2)

    in_maps = []
    for i in range(N_CORES):
        Bi = np.asarray(WT[:, i * R_SH:(i + 1) * R_SH])   # [2048, 256]
        Bi_dev = Bi.reshape(K_CH, 128, R_SH).transpose(1, 0, 2)
        merged = np.empty((128, K_CH, W_CH), dtype=np_dt)
        merged[:, :, :BATCH] = xT_dev
        merged[:, :, BATCH:] = Bi_dev
        in_maps.append({"IN": merged})
    return in_maps


def kernel(x, V, alpha):
    global LAST_RESULT
    x = np.asarray(x, dtype=np.float32)
    V = np.asarray(V, dtype=np.float32)
    alpha = np.asarray(alpha, dtype=np.float32)

    in_maps = _host_shards(x, V, alpha, DEVICE_DTYPE)
    nc = _get_graph(DEVICE_DTYPE)
    res = bass_utils.run_bass_kernel_spmd(
        nc, in_maps, core_ids=list(range(N_CORES)),
        trace=TRACE, trace_kwargs=TRACE_KWARGS)
    LAST_RESULT = res
    out = np.concatenate([np.asarray(r["out"]) for r in res.results], axis=1)
    return np.ascontiguousarray(out, dtype=np.float32)
